# revision 18
# baseline (speedup 1.0000x reference)
"""Butterworth bandpass (cascaded biquad IIR) Trainium2 kernel, v2.

Problem: y = sosfilt(sos, x) over x[32, 64, 4096] fp32 -- 2048 independent
signals, 4 cascaded DF2T biquads, sequential over T=4096.

Strategy (exact block state-space reformulation, fp16 operands + fp32 psum):
  The cascade is a linear state-space system (A[8,8], B, C, D).  Split T into
  blocks of L=120 so a block's input AND its 8-dim entry state stack into one
  K=128 contraction:
      y_b   = [Th^T; Z^T]^T  @ [x_b; s_b]          (one matmul per block/group)
      s_b+1 = via per-window state table matmuls (4 blocks per window share a
              [32, 256] psum; only the source-0 matmul depends on s_w, so the
              serial state chain is 9 links and hides under other PE work)
  x arrives [signal, time]; each block is PE-transposed once ([sig,128]->
  [120,sig]) and cast to fp16 during the psum->SBUF copy.  All matmul
  operands are fp16 (1 cyc/row on the PE at any N -- measured ~2x faster
  than float32r); accumulation stays fp32 in PSUM.  2048 signals are
  sharded 256 per NeuronCore (two groups of 128 partitions).
"""

import numpy as np

import concourse.bass as bass
import concourse.tile as tile
from concourse import bacc
from concourse import mybir
from concourse.bass_utils import run_bass_kernel_spmd

FP32 = mybir.dt.float32
FP32R = mybir.dt.float32r
FP16 = mybir.dt.float16

T = 4096
NCORES = 8
NSIG = 2048        # 32*64 independent signals
SPC = NSIG // NCORES   # 256 signals per core
G = 128            # partition group (2 groups of 128 signals)
NST = 8            # state dim of the 4-biquad cascade
L = 120            # block length (K = L + NST = 128)
LT = 16            # tail block length (T = 34*L + LT)
NB = 34            # full blocks; block index 34 is the tail
RW = 4             # blocks per state window
NW = 8             # full state windows (blocks 0..31)
WS = RW * L        # 480 samples per full window
TAILW = T - NW * WS  # 256 = 2*L + LT (blocks 32, 33, 34)


# ----------------------------------------------------------------------------
# host-side: derive block-filter matrices from sos
# ----------------------------------------------------------------------------

def _build_system(sos):
    """Cascade of biquads (DF2T) -> single state space (A, B, C, D), float64."""
    sos = np.asarray(sos, dtype=np.float64)
    A = np.zeros((0, 0))
    B = np.zeros((0,))
    C = np.zeros((0,))
    D = 1.0
    for (b0, b1, b2, _one, a1, a2) in sos:
        As = np.array([[-a1, 1.0], [-a2, 0.0]])
        Bs = np.array([b1 - a1 * b0, b2 - a2 * b0])
        Cs = np.array([1.0, 0.0])
        Ds = b0
        n = A.shape[0]
        Anew = np.zeros((n + 2, n + 2))
        Anew[:n, :n] = A
        Anew[n:, :n] = np.outer(Bs, C)
        Anew[n:, n:] = As
        A = Anew
        B = np.concatenate([B, Bs * D])
        C = np.concatenate([Ds * C, Cs])
        D = Ds * D
    return A, B, C, D


def _balance(A, B, C):
    """Square-root balanced realization: keeps state magnitudes O(1) so the
    fp16 state quantization noise stays small relative to the O(1) output."""
    P = np.outer(B, B)
    Ak = A.copy()
    for _ in range(64):
        P = P + Ak @ P @ Ak.T
        Ak = Ak @ Ak
    Q = np.outer(C, C)
    Ak = A.copy()
    for _ in range(64):
        Q = Q + Ak.T @ Q @ Ak
        Ak = Ak @ Ak
    Rc = np.linalg.cholesky(P + 1e-30 * np.eye(len(B)))
    M = Rc.T @ Q @ Rc
    lam, U = np.linalg.eigh(M)
    lam = np.maximum(lam, 1e-30)
    Tm = Rc @ U @ np.diag(lam ** -0.25)
    Ti = np.diag(lam ** 0.25) @ U.T @ np.linalg.inv(Rc)
    return Ti @ A @ Tm, Ti @ B, C @ Tm


def _build_tables(sos):
    """fp16 operator tables.

    TZ  [128, 120]: rows 0:120 = Th^T, rows 120:128 = Z^T
                    (y_b = Th x_b + Z s_b as one K=128 matmul per group)
    SF  [128, 128]: col-block r (32 cols, targets j=0..3 meaning s_{b0+j+1}):
                    rows 0:120 = (A_L^(j-r) F)^T for j >= r; rows 120:128 =
                    (A_L^(j+1))^T for r == 0 (state recursion), else 0
    TAIL[128, 48]:  cols 0:16 tail-window src0, cols 16:32 src1 (targets
                    s_33, s_34), cols 32:48 rows 0:24 = tail TZ [LT+8, LT]
    """
    A, B, C, D = _build_system(sos)
    A, B, C = _balance(A, B, C)

    def conv_tables(Lb):
        h = np.zeros(Lb)
        h[0] = D
        An = np.eye(NST)
        for k in range(1, Lb):
            h[k] = C @ An @ B
            An = An @ A
        Th = np.zeros((Lb, Lb))
        for m in range(Lb):
            Th[m:, m] = h[: Lb - m]      # Th[t, t'] = h[t - t']
        Z = np.zeros((Lb, NST))
        CAn = C.copy()
        for t in range(Lb):
            Z[t] = CAn
            CAn = CAn @ A
        F = np.zeros((NST, Lb))          # F[:, t'] = A^(Lb-1-t') B
        AmB = B.copy()
        for t in range(Lb - 1, -1, -1):
            F[:, t] = AmB
            AmB = A @ AmB
        AL = np.linalg.matrix_power(A, Lb)
        return Th, Z, F, AL

    Th, Z, F, AL = conv_tables(L)
    ThT, ZT, _, _ = conv_tables(LT)

    TZ = np.concatenate([Th.T, Z.T], axis=0)            # [128, 120]
    TZtail = np.concatenate([ThT.T, ZT.T], axis=0)      # [24, 16]

    ALp = [np.eye(NST)]
    for _ in range(RW):
        ALp.append(ALp[-1] @ AL)

    # stage-row order: slot 0 = s_{b0+4} (next window's entry state, feeds the
    # chain matmul directly from partitions 0:8), slots 1..3 = s_{b0+1..3}
    TGT = [4, 1, 2, 3]
    SF = np.zeros((L, 128))                # x-part, K=120 (state part split off)
    for r in range(RW):
        cb = 32 * r
        for sl, jt in enumerate(TGT):
            if jt - 1 >= r:
                SF[:, cb + 8 * sl:cb + 8 * sl + 8] = (ALp[jt - 1 - r] @ F).T
    SA = np.zeros((NST, 48))               # state recursion, rhs = stage[0:8]
    for sl, jt in enumerate(TGT):
        SA[:, 8 * sl:8 * sl + 8] = ALp[jt].T
    # tail window: slots [s_33, s_34]; recursion from s_32 (= stage slot 0)
    for jt in (1, 2):
        SA[:, 32 + 8 * (jt - 1):32 + 8 * jt] = ALp[jt].T

    TAIL = np.zeros((G, 48))
    for r in range(2):
        cb = 16 * r
        for j in range(r, 2):
            TAIL[0:L, cb + 8 * j:cb + 8 * j + 8] = (ALp[j - r] @ F).T
    TAIL[0:LT + NST, 32:48] = TZtail

    f16 = lambda a: np.ascontiguousarray(a, dtype=np.float16)
    return f16(TZ), f16(SF), f16(SA), f16(TAIL)


# ----------------------------------------------------------------------------
# device kernel
# ----------------------------------------------------------------------------

def _build_nc():
    nc = bacc.Bacc("TRN2", target_bir_lowering=False)
    x_d = nc.dram_tensor("x", [SPC, T], FP32R, kind="ExternalInput").ap()
    ident_d = nc.dram_tensor("ident", [G, G], FP32R, kind="ExternalInput").ap()
    tz_d = nc.dram_tensor("tz", [G, L], FP16, kind="ExternalInput").ap()
    sf_d = nc.dram_tensor("sf", [L, G], FP16, kind="ExternalInput").ap()
    sa_d = nc.dram_tensor("sa", [NST, 48], FP16, kind="ExternalInput").ap()
    tail_d = nc.dram_tensor("tailt", [G, 48], FP16, kind="ExternalInput").ap()
    zer_d = nc.dram_tensor("zer", [NST, 2 * G], FP16, kind="ExternalInput").ap()
    y_d = nc.dram_tensor("y", [SPC, T], FP32, kind="ExternalOutput").ap()

    with tile.TileContext(nc) as tc:
        with (
            tc.tile_pool(name="consts", bufs=1) as consts,
            tc.tile_pool(name="xin", bufs=1) as xin,
            tc.tile_pool(name="xstk", bufs=1) as xstk,
            tc.tile_pool(name="yout", bufs=1) as yout,
            tc.tile_pool(name="pxt", bufs=2, space="PSUM") as pxt,
            tc.tile_pool(name="ps", bufs=2, space="PSUM") as psp,
            tc.tile_pool(name="py", bufs=2, space="PSUM") as pyp,
        ):
            # window-0 x loads first (gate the first transposes)
            xw = []
            for w in range(NW + 1):
                cols = WS if w < NW else TAILW
                pair = []
                for g in (0, 1):
                    t_ = xin.tile([G, cols], FP32R, tag=f"x{w}g{g}",
                                  name=f"x_w{w}g{g}")
                    nc.sync.dma_start(
                        t_, x_d[g * G:(g + 1) * G, w * WS:w * WS + cols])
                    pair.append(t_)
                xw.append(pair)

            ident = consts.tile([G, G], FP32R, tag="ident")
            nc.sync.dma_start(ident, ident_d)
            tz_sb = consts.tile([G, L], FP16, tag="tz")
            nc.sync.dma_start(tz_sb, tz_d)
            sf_sb = consts.tile([L, G], FP16, tag="sf")
            nc.sync.dma_start(sf_sb, sf_d)
            sa_sb = consts.tile([NST, 48], FP16, tag="sa")
            nc.sync.dma_start(sa_sb, sa_d)
            tail_sb = consts.tile([G, 48], FP16, tag="tail")
            nc.sync.dma_start(tail_sb, tail_d)

            # stacked tiles: rows 0:L = x_b^T (fp16), rows L:L+8 = s_b
            X = [
                xstk.tile([G, 2 * G], FP16, tag=f"X{b}", name=f"X{b}")
                for b in range(NB)
            ]
            X.append(xstk.tile([LT + NST, 2 * G], FP16, tag="X34", name="X34"))
            # s_0 = 0
            nc.sync.dma_start(X[0][L:L + NST, :], zer_d)

            def transpose_block(b, w, r, blen):
                """x window w, block-in-window r -> X[b] rows 0:blen (fp16)."""
                psum_t = pxt.tile([L, 2 * G], FP32R, tag="pxt",
                                  name=f"pst{b}")
                for g in (0, 1):
                    nc.tensor.transpose(
                        psum_t[0:blen, g * G:(g + 1) * G],
                        xw[w][g][:, r * L:r * L + blen],
                        ident,
                    )
                nc.vector.tensor_copy(X[b][0:blen, :], psum_t[0:blen, :])

            def y_block(b, psum_y, coff, blen, rhs):
                """y matmuls for block b into psum_y[:, coff:coff+blen]."""
                kk = blen + NST
                for g in (0, 1):
                    nc.tensor.matmul(
                        psum_y[g][:, coff:coff + blen],
                        X[b][0:kk, g * G:(g + 1) * G],
                        rhs,
                        start=True, stop=True,
                    )

            for w in range(NW + 1):
                full = w < NW
                nblk = RW if full else 2
                b0 = RW * w

                for r in range(nblk):
                    transpose_block(b0 + r, w, r, L)
                if not full:
                    transpose_block(34, w, 2, LT)

                # state window: x-part matmuls (K=120, no state dependency)
                # plus one recursion matmul from the previous stage tile.
                # stage-slot order (full window): [s_{b0+4} | s_{b0+1..3}]
                ntgt = RW if full else 2
                psum_s = psp.tile([RW * NST, 2 * G], FP32, tag="ps",
                                  name=f"ps{w}")
                stab = sf_sb if full else tail_sb
                scw = 32 if full else 16
                for r in range(nblk):
                    nc.tensor.matmul(
                        psum_s[0:ntgt * NST, :],
                        stab[0:L, scw * r:scw * (r + 1)],
                        X[b0 + r][0:L, :],
                        start=(r == 0), stop=(r == nblk - 1 and w == 0),
                    )
                if w > 0:
                    nc.tensor.matmul(
                        psum_s[0:ntgt * NST, :],
                        sa_sb[:, 0:ntgt * NST] if full else sa_sb[:, 32:48],
                        stage_prev[0:NST, :],
                        start=False, stop=True,
                    )
                stage = xstk.tile([RW * NST, 2 * G], FP16, tag=f"st{w}",
                                  name=f"stage{w}")
                nc.vector.tensor_copy(stage[0:ntgt * NST, :],
                                      psum_s[0:ntgt * NST, :])
                stage_prev = stage
                # scatter states into the stacked tiles (cross-partition
                # moves are DMA-only; engines cannot shift partitions)
                dq = [nc.sync, nc.scalar]
                if full:
                    sl_tb = [(0, b0 + 4), (1, b0 + 1), (2, b0 + 2), (3, b0 + 3)]
                else:
                    sl_tb = [(0, 33), (1, 34)]
                for i, (sl, tb) in enumerate(sl_tb):
                    dst = X[tb][L:L + NST, :] if tb < NB else \
                        X[34][LT:LT + NST, :]
                    dq[i % 2].dma_start(dst, stage[NST * sl:NST * (sl + 1), :])

                # y matmuls + copy-out + DMA
                wcols = WS if full else TAILW
                psum_y = [
                    pyp.tile([G, wcols], FP32, tag=f"py{g}", name=f"py{w}g{g}",
                             padded_shape=[G, WS])
                    for g in (0, 1)
                ]
                for r in range(nblk):
                    y_block(b0 + r, psum_y, r * L, L, tz_sb[:, 0:L])
                if not full:
                    y_block(34, psum_y, 2 * L, LT, tail_sb[0:LT + NST, 32:48])

                y_sb = [
                    yout.tile([G, wcols], FP32, tag=f"y{w}g{g}",
                              name=f"y_w{w}g{g}")
                    for g in (0, 1)
                ]
                for g in (0, 1):
                    if g == 0:
                        nc.vector.tensor_copy(y_sb[g], psum_y[g])
                    else:
                        nc.scalar.copy(y_sb[g], psum_y[g])
                    nc.scalar.dma_start(
                        y_d[g * G:(g + 1) * G, w * WS:w * WS + wcols],
                        y_sb[g],
                    )
    nc.compile()
    return nc


_NC_CACHE = None
LAST_RESULTS = None  # BassKernelResults of the most recent kernel() call


def _get_nc():
    global _NC_CACHE
    if _NC_CACHE is None:
        _NC_CACHE = _build_nc()
    return _NC_CACHE


def kernel(x: np.ndarray, sos: np.ndarray) -> np.ndarray:
    x = np.asarray(x)
    orig_shape = x.shape
    orig_dtype = x.dtype
    TZ, SF, SA, TAIL = _build_tables(np.asarray(sos, dtype=np.float64))

    xf = np.ascontiguousarray(x.reshape(NSIG, T), dtype=np.float32)
    ident = np.eye(G, dtype=np.float32)
    zer = np.zeros((NST, 2 * G), dtype=np.float16)
    in_maps = [
        {"x": xf[c * SPC:(c + 1) * SPC], "ident": ident, "tz": TZ, "sf": SF,
         "sa": SA, "tailt": TAIL, "zer": zer}
        for c in range(NCORES)
    ]
    nc = _get_nc()
    res = run_bass_kernel_spmd(nc, in_maps, core_ids=list(range(NCORES)))
    global LAST_RESULTS
    LAST_RESULTS = res
    y = np.concatenate([res.results[c]["y"] for c in range(NCORES)], axis=0)
    return y.reshape(orig_shape).astype(orig_dtype, copy=False)


# revision 25
# speedup vs baseline: 1.1319x; 1.1319x over previous
"""Butterworth bandpass (cascaded biquad IIR) Trainium2 kernel, v3.

Problem: y = sosfilt(sos, x) over x[32, 64, 4096] fp32 -- 2048 independent
signals, 4 cascaded DF2T biquads, sequential over T=4096.

Strategy (exact block state-space reformulation, fp16 operands + fp32 psum):
  The cascade is a linear state-space system (A[8,8], B, C, D).  Split T into
  32 blocks of L=128, grouped in windows of 4:
      y_b   = Th x_b + Z s_b      (per-block conv matmul + state correction)
      s_.   = window state table matmuls into one psum; only the recursion
              matmul depends on s_w, so the serial state chain is 8 links
              and hides under the rest of the PE work
  Window-entry states live in a partition-banded "stage" tile (slot for
  s_{b0+4} at partitions 0:8, slots for s_{b0+1..3} at partitions 32:56)
  because matmul operands must start at partition 0/32/64.  The three
  intra-window corrections are one block-diagonal [24, 384] matmul; the
  block-0 correction reads the previous window's stage at partitions 0:8.
  x arrives [signal, time]; each block is PE-transposed once and cast to
  fp16 during the psum->SBUF copy.  All matmul operands are fp16 (1 cyc/row
  on the PE at any N -- measured ~2x faster than float32r); accumulation
  stays fp32 in PSUM.  2048 signals are sharded 256 per NeuronCore (two
  groups of 128 partitions).
"""

import numpy as np

import concourse.bass as bass
import concourse.tile as tile
from concourse import bacc
from concourse import mybir
from concourse.bass_utils import run_bass_kernel_spmd

FP32 = mybir.dt.float32
FP32R = mybir.dt.float32r
FP16 = mybir.dt.float16

T = 4096
NCORES = 8
NSIG = 2048        # 32*64 independent signals
SPC = NSIG // NCORES   # 256 signals per core
G = 128            # partition group (2 groups of 128 signals)
NST = 8            # state dim of the 4-biquad cascade
L = 128            # block length
RW = 4             # blocks per state window
NW = 8             # state windows
WS = RW * L        # 512 samples per window
NTGT = 56          # state-psum partition span: s4 at 0:8, s1..s3 at 32:56


# ----------------------------------------------------------------------------
# host-side: derive block-filter matrices from sos
# ----------------------------------------------------------------------------

def _build_system(sos):
    """Cascade of biquads (DF2T) -> single state space (A, B, C, D), float64."""
    sos = np.asarray(sos, dtype=np.float64)
    A = np.zeros((0, 0))
    B = np.zeros((0,))
    C = np.zeros((0,))
    D = 1.0
    for (b0, b1, b2, _one, a1, a2) in sos:
        As = np.array([[-a1, 1.0], [-a2, 0.0]])
        Bs = np.array([b1 - a1 * b0, b2 - a2 * b0])
        Cs = np.array([1.0, 0.0])
        Ds = b0
        n = A.shape[0]
        Anew = np.zeros((n + 2, n + 2))
        Anew[:n, :n] = A
        Anew[n:, :n] = np.outer(Bs, C)
        Anew[n:, n:] = As
        A = Anew
        B = np.concatenate([B, Bs * D])
        C = np.concatenate([Ds * C, Cs])
        D = Ds * D
    return A, B, C, D


def _balance(A, B, C):
    """Square-root balanced realization: keeps state magnitudes O(1) so the
    fp16 state quantization noise stays small relative to the O(1) output."""
    P = np.outer(B, B)
    Ak = A.copy()
    for _ in range(64):
        P = P + Ak @ P @ Ak.T
        Ak = Ak @ Ak
    Q = np.outer(C, C)
    Ak = A.copy()
    for _ in range(64):
        Q = Q + Ak.T @ Q @ Ak
        Ak = Ak @ Ak
    Rc = np.linalg.cholesky(P + 1e-30 * np.eye(len(B)))
    M = Rc.T @ Q @ Rc
    lam, U = np.linalg.eigh(M)
    lam = np.maximum(lam, 1e-30)
    Tm = Rc @ U @ np.diag(lam ** -0.25)
    Ti = np.diag(lam ** 0.25) @ U.T @ np.linalg.inv(Rc)
    return Ti @ A @ Tm, Ti @ B, C @ Tm


def _build_tables(sos):
    """fp16 operator tables (slot layout: s_{b0+4} at rows/cols 0:8,
    s_{b0+k} at 32+8(k-1) for k=1..3).

    THT [128, 128]: Th^T             (conv rhs: y_b[sig, t] = x_b Th^T)
    SF  [128, 4*56]: x-part state tables, source block r in col-block r
    SA  [8, 56]:    state recursion table (rhs = prev stage rows 0:8)
    CORR[56, 384]:  rows 0:8 cols 0:128 = Z^T (block-0 correction);
                    rows 32:56 = blockdiag(Z^T x3) for blocks 1..3
                    (cols 0:384 map to y cols 128:512)
    """
    A, B, C, D = _build_system(sos)
    A, B, C = _balance(A, B, C)

    h = np.zeros(L)
    h[0] = D
    An = np.eye(NST)
    for k in range(1, L):
        h[k] = C @ An @ B
        An = An @ A
    Th = np.zeros((L, L))
    for m in range(L):
        Th[m:, m] = h[: L - m]       # Th[t, t'] = h[t - t']
    Z = np.zeros((L, NST))
    CAn = C.copy()
    for t in range(L):
        Z[t] = CAn
        CAn = CAn @ A
    F = np.zeros((NST, L))           # F[:, t'] = A^(L-1-t') B
    AmB = B.copy()
    for t in range(L - 1, -1, -1):
        F[:, t] = AmB
        AmB = A @ AmB
    AL = np.linalg.matrix_power(A, L)

    ALp = [np.eye(NST)]
    for _ in range(RW):
        ALp.append(ALp[-1] @ AL)

    slot_col = {4: 0, 1: 32, 2: 40, 3: 48}   # target -> column in state psum

    SF = np.zeros((L, RW * NTGT))
    for r in range(RW):
        cb = NTGT * r
        for jt in (1, 2, 3, 4):
            if jt - 1 >= r:
                c = cb + slot_col[jt]
                SF[:, c:c + NST] = (ALp[jt - 1 - r] @ F).T
    SA = np.zeros((NST, NTGT))
    for jt in (1, 2, 3, 4):
        c = slot_col[jt]
        SA[:, c:c + NST] = ALp[jt].T

    CORR = np.zeros((NTGT, 3 * L))
    CORR[0:NST, 0:L] = Z.T           # only cols 0:128 used for this row band
    for k in (1, 2, 3):
        rb = slot_col[k]
        CORR[rb:rb + NST, (k - 1) * L:k * L] = Z.T

    f16 = lambda a: np.ascontiguousarray(a, dtype=np.float16)
    return f16(Th.T), f16(SF), f16(SA), f16(CORR)


# ----------------------------------------------------------------------------
# device kernel
# ----------------------------------------------------------------------------

def _build_nc():
    nc = bacc.Bacc("TRN2", target_bir_lowering=False)
    x_d = nc.dram_tensor("x", [SPC, T], FP32R, kind="ExternalInput").ap()
    ident_d = nc.dram_tensor("ident", [G, G], FP32R, kind="ExternalInput").ap()
    tht_d = nc.dram_tensor("tht", [L, L], FP16, kind="ExternalInput").ap()
    sf_d = nc.dram_tensor("sf", [L, RW * NTGT], FP16,
                          kind="ExternalInput").ap()
    sa_d = nc.dram_tensor("sa", [NST, NTGT], FP16, kind="ExternalInput").ap()
    corr_d = nc.dram_tensor("corr", [NTGT, 3 * L], FP16,
                            kind="ExternalInput").ap()
    y_d = nc.dram_tensor("y", [SPC, T], FP32, kind="ExternalOutput").ap()

    with tile.TileContext(nc) as tc:
        with (
            tc.tile_pool(name="consts", bufs=1) as consts,
            tc.tile_pool(name="xin", bufs=1) as xin,
            tc.tile_pool(name="xstk", bufs=1) as xstk,
            tc.tile_pool(name="yout", bufs=1) as yout,
            tc.tile_pool(name="pxt", bufs=2, space="PSUM") as pxt,
            tc.tile_pool(name="ps", bufs=2, space="PSUM") as psp,
            tc.tile_pool(name="py", bufs=2, space="PSUM") as pyp,
        ):
            # consts lead the sync queue (small, gate all compute); the big
            # x loads for windows 0/1 follow, later windows paced in-loop
            ident = consts.tile([G, G], FP32R, tag="ident")
            nc.sync.dma_start(ident, ident_d)
            tht_sb = consts.tile([L, L], FP16, tag="tht")
            sf_sb = consts.tile([L, RW * NTGT], FP16, tag="sf")
            sa_sb = consts.tile([NST, NTGT], FP16, tag="sa")
            corr_sb = consts.tile([NTGT, 3 * L], FP16, tag="corr")
            nc.scalar.dma_start(sf_sb, sf_d)
            nc.scalar.dma_start(sa_sb, sa_d)
            nc.scalar.dma_start(corr_sb, corr_d)

            xw = []
            for w in range(NW):
                xw.append([
                    xin.tile([G, WS], FP32R, tag=f"x{w}g{g}",
                             name=f"x_w{w}g{g}")
                    for g in (0, 1)
                ])

            def load_x(w):
                for g in (0, 1):
                    nc.sync.dma_start(
                        xw[w][g], x_d[g * G:(g + 1) * G, w * WS:(w + 1) * WS])

            load_x(0)
            nc.sync.dma_start(tht_sb, tht_d)
            load_x(1)

            # transposed x blocks, fp16 [time, sig]
            X = [
                xstk.tile([L, 2 * G], FP16, tag=f"X{b}", name=f"X{b}")
                for b in range(RW * NW)
            ]

            def transpose_block(b):
                psum_t = pxt.tile([L, 2 * G], FP32R, tag="pxt", name=f"pst{b}")
                for g in (0, 1):
                    nc.tensor.transpose(
                        psum_t[:, g * G:(g + 1) * G],
                        xw[b // RW][g][:, (b % RW) * L:(b % RW + 1) * L],
                        ident,
                    )
                nc.vector.tensor_copy(X[b], psum_t)

            def state_mm(w, r, psum_s, stage_of_prev):
                """r in 0..RW-1 -> x-part; r == RW -> recursion matmul."""
                if r < RW:
                    nc.tensor.matmul(
                        psum_s, sf_sb[:, NTGT * r:NTGT * (r + 1)],
                        X[RW * w + r],
                        start=(r == 0), stop=(r == RW - 1 and w == 0),
                    )
                elif w > 0:
                    nc.tensor.matmul(
                        psum_s, sa_sb, stage_of_prev[0:NST, :],
                        start=False, stop=True,
                    )

            # prologue: window 0 state path
            stages = []
            for r in range(RW):
                transpose_block(r)
            psum_s0 = psp.tile([NTGT, 2 * G], FP32, tag="ps", name="ps0")
            for r in range(RW + 1):
                state_mm(0, r, psum_s0, None)
            st0 = xstk.tile([NTGT, 2 * G], FP16, tag="st0", name="stage0")
            nc.vector.tensor_copy(st0, psum_s0)
            stages.append(st0)

            # steady state: y path for window w, state path for window w+1
            for w in range(NW):
                b0 = RW * w
                nxt = w + 1 < NW
                if w + 2 < NW:
                    load_x(w + 2)

                psum_y = [
                    pyp.tile([G, WS], FP32, tag=f"py{g}", name=f"py{w}g{g}")
                    for g in (0, 1)
                ]
                if nxt:
                    for r in range(RW):
                        transpose_block(b0 + RW + r)
                # one fully contiguous accumulation group per psum_y bank:
                # convs then corrections, no other matmul in between
                for g in (0, 1):
                    for r in range(RW):
                        nc.tensor.matmul(
                            psum_y[g][:, r * L:(r + 1) * L],
                            X[b0 + r][:, g * G:(g + 1) * G],
                            tht_sb,
                            start=(r == 0), stop=False,
                        )
                    if w > 0:
                        nc.tensor.matmul(
                            psum_y[g][:, 0:L],
                            stages[w - 1][0:NST, g * G:(g + 1) * G],
                            corr_sb[0:NST, 0:L],
                            start=False, stop=False,
                        )
                    nc.tensor.matmul(
                        psum_y[g][:, L:WS],
                        stages[w][32:NTGT, g * G:(g + 1) * G],
                        corr_sb[32:NTGT, :],
                        start=False, stop=True,
                    )
                if nxt:
                    psum_s = psp.tile([NTGT, 2 * G], FP32, tag="ps",
                                      name=f"ps{w + 1}")
                    for i in range(RW + 1):
                        state_mm(w + 1, i, psum_s,
                                 stages[w] if i == RW else None)
                    st = xstk.tile([NTGT, 2 * G], FP16, tag=f"st{w + 1}",
                                   name=f"stage{w + 1}")
                    nc.vector.tensor_copy(st, psum_s)
                    stages.append(st)

                y_sb = [
                    yout.tile([G, WS], FP32, tag=f"y{w}g{g}",
                              name=f"y_w{w}g{g}")
                    for g in (0, 1)
                ]
                for g in (0, 1):
                    if g == 0:
                        nc.vector.tensor_copy(y_sb[g], psum_y[g])
                    else:
                        nc.scalar.copy(y_sb[g], psum_y[g])
                    nc.scalar.dma_start(
                        y_d[g * G:(g + 1) * G, w * WS:(w + 1) * WS],
                        y_sb[g],
                    )
    nc.compile()
    return nc


_NC_CACHE = None
LAST_RESULTS = None  # BassKernelResults of the most recent kernel() call


def _get_nc():
    global _NC_CACHE
    if _NC_CACHE is None:
        _NC_CACHE = _build_nc()
    return _NC_CACHE


def kernel(x: np.ndarray, sos: np.ndarray) -> np.ndarray:
    x = np.asarray(x)
    orig_shape = x.shape
    orig_dtype = x.dtype
    THT, SF, SA, CORR = _build_tables(np.asarray(sos, dtype=np.float64))

    xf = np.ascontiguousarray(x.reshape(NSIG, T), dtype=np.float32)
    ident = np.eye(G, dtype=np.float32)
    in_maps = [
        {"x": xf[c * SPC:(c + 1) * SPC], "ident": ident, "tht": THT,
         "sf": SF, "sa": SA, "corr": CORR}
        for c in range(NCORES)
    ]
    nc = _get_nc()
    res = run_bass_kernel_spmd(nc, in_maps, core_ids=list(range(NCORES)))
    global LAST_RESULTS
    LAST_RESULTS = res
    y = np.concatenate([res.results[c]["y"] for c in range(NCORES)], axis=0)
    return y.reshape(orig_shape).astype(orig_dtype, copy=False)


# revision 31
# speedup vs baseline: 1.1699x; 1.0335x over previous
"""Butterworth bandpass (cascaded biquad IIR) Trainium2 kernel.

Problem: y = sosfilt(sos, x) over x[32, 64, 4096] fp32 -- 2048 independent
signals, 4 cascaded DF2T biquads, sequential over T=4096.

Strategy (exact block-parallel reformulation, no truncation):
  The cascade is a linear state-space system (A[8,8], B, C, D).  Split T into
  blocks of L=128, grouped in windows of R=4 blocks.  With s = state at the
  window entry, for block r of the window (all operators precomputed on host
  in float64 from the 24 sos coefficients):
      y_r = Th @ x_r + sum_{r'<r} (Z A_L^{r-r'-1} F) @ x_{r'} + (Z A_L^r) @ s
      s'  = A_L^R @ s + sum_r (A_L^{R-1-r} F) @ x_r
  On device everything is TensorE matmuls over [signal, time] tiles:
    - per block, transpose x[sig, time] -> xT[time, sig] on the PE;
    - one fused rhs table THW[128, 512] = [Th | ZF | ZA_LF | ZA_L^2F] turns
      conv + all intra-window cross-block corrections into a single
      accumulated matmul per source block (lhsT = xT_r, N = 512-128r);
    - entry-state corrections for all 4 blocks come from one matmul with
      rhs ZA[8, 512] (lhsT = s);
    - the state update accumulates in a [8, 256] psum.
  Matmul operands use dtype float32r (single-pass fp32 PE mode, 1 cyc/row at
  N>=256 vs 4 cyc/row for fp32 LOW_HIGH).  Conv outputs land directly in
  [signal, time] layout, so no output transpose is needed.  2048 signals are
  sharded 256 per NeuronCore (two groups of 128 output partitions).
"""

import numpy as np

import concourse.bass as bass
import concourse.tile as tile
from concourse import bacc
from concourse import mybir
from concourse.bass_utils import run_bass_kernel_spmd

FP32 = mybir.dt.float32
FP32R = mybir.dt.float32r
FP16 = mybir.dt.float16

P = 128            # partition width == time-block length
T = 4096
NCORES = 8
NSIG = 2048        # 32*64 independent signals
SPC = NSIG // NCORES   # 256 signals per core
NST = 8            # state dim of the 4-biquad cascade
R = 4              # blocks per window
W = P * R          # 512 time steps per window (== DMA chunk)
NW = T // W        # 8 windows


# ----------------------------------------------------------------------------
# host-side: derive block-filter matrices from sos
# ----------------------------------------------------------------------------

def _build_system(sos):
    """Cascade of biquads (DF2T) -> single state space (A, B, C, D), float64."""
    sos = np.asarray(sos, dtype=np.float64)
    A = np.zeros((0, 0))
    B = np.zeros((0,))
    C = np.zeros((0,))
    D = 1.0
    for (b0, b1, b2, _one, a1, a2) in sos:
        As = np.array([[-a1, 1.0], [-a2, 0.0]])
        Bs = np.array([b1 - a1 * b0, b2 - a2 * b0])
        Cs = np.array([1.0, 0.0])
        Ds = b0
        n = A.shape[0]
        Anew = np.zeros((n + 2, n + 2))
        Anew[:n, :n] = A
        Anew[n:, :n] = np.outer(Bs, C)
        Anew[n:, n:] = As
        A = Anew
        B = np.concatenate([B, Bs * D])
        C = np.concatenate([Ds * C, Cs])
        D = Ds * D
    return A, B, C, D


def _balance(A, B, C):
    """Square-root balanced realization: both gramians become diagonal and
    equal, minimizing intermediate-magnitude disparity (important because the
    PE's float32r mode rounds products; unbalanced states reach |s|~650 and
    the rounding noise then dwarfs the O(1) output)."""
    P = np.outer(B, B)
    Ak = A.copy()
    for _ in range(64):
        P = P + Ak @ P @ Ak.T
        Ak = Ak @ Ak
    Q = np.outer(C, C)
    Ak = A.copy()
    for _ in range(64):
        Q = Q + Ak.T @ Q @ Ak
        Ak = Ak @ Ak
    Rc = np.linalg.cholesky(P + 1e-30 * np.eye(len(B)))
    M = Rc.T @ Q @ Rc
    lam, U = np.linalg.eigh(M)
    lam = np.maximum(lam, 1e-30)
    Tm = Rc @ U @ np.diag(lam ** -0.25)
    Ti = np.diag(lam ** 0.25) @ U.T @ np.linalg.inv(Rc)
    return Ti @ A @ Tm, Ti @ B, C @ Tm


def _build_matrices(sos):
    """Window-fused operator tables, all fp32 (fed to float32r device tiles).

    THW[128, 512]: cols [128d:128d+128] = Th (d=0) or (Z A_L^(d-1) F)^T (d>=1)
    ZA [8, 512]:   cols [128r:128r+128] = (Z A_L^r)^T
    FTR[128, 32]:  cols [8r:8r+8]       = ((A_L^(R-1-r)) F)^T
    A4T[8, 8]:     (A_L^R)^T
    """
    A, B, C, D = _build_system(sos)
    A, B, C = _balance(A, B, C)
    ns = A.shape[0]
    assert ns == NST

    h = np.zeros(P)
    h[0] = D
    An = np.eye(ns)
    for k in range(1, P):
        h[k] = C @ An @ B
        An = An @ A
    Th = np.zeros((P, P))
    for m in range(P):
        Th[m, m:] = h[: P - m]

    Z = np.zeros((P, ns))
    CAn = C.copy()
    for n in range(P):
        Z[n] = CAn
        CAn = CAn @ A

    F = np.zeros((ns, P))
    AmB = B.copy()
    for m in range(P - 1, -1, -1):
        F[:, m] = AmB
        AmB = A @ AmB

    AL = np.linalg.matrix_power(A, P)

    THW = np.zeros((P, R * P))
    THW[:, :P] = Th
    for d in range(1, R):
        THW[:, d * P:(d + 1) * P] = (Z @ np.linalg.matrix_power(AL, d - 1) @ F).T
    ZA = np.zeros((ns, R * P))
    for r in range(R):
        ZA[:, r * P:(r + 1) * P] = (Z @ np.linalg.matrix_power(AL, r)).T
    FTR = np.zeros((P, R * NST))
    for r in range(R):
        FTR[:, r * NST:(r + 1) * NST] = (np.linalg.matrix_power(AL, R - 1 - r) @ F).T
    A4T = np.linalg.matrix_power(AL, R).T

    f32 = lambda a: np.ascontiguousarray(a, dtype=np.float32)
    return f32(THW), f32(ZA), f32(FTR), f32(A4T)


# ----------------------------------------------------------------------------
# device kernel
# ----------------------------------------------------------------------------

def _build_nc():
    nc = bacc.Bacc("TRN2", target_bir_lowering=False)
    x_d = nc.dram_tensor("x", [SPC, T], FP32R, kind="ExternalInput").ap()
    ident_d = nc.dram_tensor("ident", [P, P], FP32R, kind="ExternalInput").ap()
    ctab_d = nc.dram_tensor("ctab", [P, R * P + R * NST], FP16,
                            kind="ExternalInput").ap()
    ctab8_d = nc.dram_tensor("ctab8", [NST, R * P + NST + 2 * P], FP16,
                             kind="ExternalInput").ap()
    y_d = nc.dram_tensor("y", [SPC, T], FP32, kind="ExternalOutput").ap()

    with tile.TileContext(nc) as tc:
        with (
            tc.tile_pool(name="consts", bufs=1) as consts,
            tc.tile_pool(name="xpool", bufs=1) as xpool,
            tc.tile_pool(name="ypool", bufs=3) as ypool,
            tc.tile_pool(name="xtpool", bufs=8) as xtpool,
            tc.tile_pool(name="spool", bufs=4) as spool,
            tc.tile_pool(name="pxt", bufs=3, space="PSUM") as pxt,
            tc.tile_pool(name="py", bufs=2, space="PSUM") as pyp,
            tc.tile_pool(name="ps", bufs=2, space="PSUM") as psp,
        ):
            # Per-DMA-instruction throughput is low (~25-60 GB/s); aggregate
            # bandwidth comes from CONCURRENT DMA instructions.  So: all x
            # windows live in static tiles and are issued upfront, split in
            # half per group across two issue queues; the first two windows
            # are split into per-block quarters so the first transposes can
            # start as soon as their 128 columns land.
            ident = consts.tile([P, P], FP32R)
            nc.sync.dma_start(ident, ident_d)
            xw = []
            for w in range(NW):
                xw.append([
                    xpool.tile([P, W], FP32R, tag=f"x{w}g{g}",
                               name=f"x_sb{w}g{g}")
                    for g in (0, 1)
                ])
            xq = [nc.sync, nc.gpsimd]
            for w in (0, 1):
                for q in range(R):
                    for g in (0, 1):
                        xq[g].dma_start(
                            xw[w][g][:, q * P:(q + 1) * P],
                            x_d[g * P:(g + 1) * P,
                                w * W + q * P:w * W + (q + 1) * P],
                        )
            ctab_sb = consts.tile([P, R * P + R * NST], FP16)
            nc.scalar.dma_start(ctab_sb, ctab_d)
            thw_sb = ctab_sb[:, 0:R * P]
            ftr_sb = ctab_sb[:, R * P:]
            ctab8_sb = consts.tile([NST, R * P + NST], FP16)
            nc.scalar.dma_start(ctab8_sb, ctab8_d[:, :R * P + NST])
            za_sb = ctab8_sb[:, 0:R * P]
            a4t_sb = ctab8_sb[:, R * P:]

            s_prev = spool.tile([NST, 2 * P], FP16, tag="s")
            nc.scalar.dma_start(s_prev, ctab8_d[:, R * P + NST:])

            H = W // 2
            for w in range(2, NW):
                for g in (0, 1):
                    for h in (0, 1):
                        xq[g].dma_start(
                            xw[w][g][:, h * H:(h + 1) * H],
                            x_d[g * P:(g + 1) * P,
                                w * W + h * H:w * W + (h + 1) * H],
                        )

            for w in range(NW):
                x_sb = xw[w]
                y_sb = [
                    ypool.tile([P, W], FP32, tag=f"y{g}", name=f"y_sb{g}")
                    for g in (0, 1)
                ]

                # transpose the 4 blocks; xt_sb[r] = [time, sig(256)]
                xt_sb = []
                for r in range(R):
                    psum_t = pxt.tile([P, 2 * P], FP32R, tag="pxt", name=f"pst{r}")
                    for g in (0, 1):
                        nc.tensor.transpose(
                            psum_t[:, g * P:(g + 1) * P],
                            x_sb[g][:, r * P:(r + 1) * P],
                            ident,
                        )
                    xt = xtpool.tile([P, 2 * P], FP16, tag="xt", name=f"xt{r}")
                    if r % 2 == 0:
                        nc.vector.tensor_copy(xt, psum_t)
                    else:
                        nc.scalar.copy(xt, psum_t)
                    xt_sb.append(xt)

                # y accumulation: per group one [128, 512] psum bank
                psum_y = [
                    pyp.tile([P, W], FP32, tag=f"py{g}", name=f"py{g}") for g in (0, 1)
                ]
                for g in (0, 1):
                    gs = slice(g * P, (g + 1) * P)
                    nc.tensor.matmul(
                        psum_y[g], s_prev[:, gs], za_sb, start=True, stop=False,
                    )
                    for r in range(R):
                        nc.tensor.matmul(
                            psum_y[g][:, r * P:],
                            xt_sb[r][:, gs],
                            thw_sb[:, : (R - r) * P],
                            start=False, stop=(r == R - 1),
                        )

                # state update: psum_s[8, 256] over both groups
                psum_s = psp.tile([NST, 2 * P], FP32, tag="ps", bufs=1)
                nc.tensor.matmul(psum_s, a4t_sb, s_prev, start=True, stop=False)
                for r in range(R):
                    nc.tensor.matmul(
                        psum_s, ftr_sb[:, r * NST:(r + 1) * NST], xt_sb[r],
                        start=False, stop=(r == R - 1),
                    )
                s_next = spool.tile([NST, 2 * P], FP16, tag="s")
                if w % 2 == 0:
                    nc.scalar.copy(s_next, psum_s)
                else:
                    nc.vector.tensor_copy(s_next, psum_s)
                s_prev = s_next

                # write back y (half-granular) and DMA out as 4 concurrent
                # half-transfers spread over three issue queues
                yq = [nc.scalar, nc.sync, nc.scalar, nc.gpsimd]
                for g, eng in ((0, nc.vector.tensor_copy), (1, nc.scalar.copy)):
                    for h in (0, 1):
                        eng(y_sb[g][:, h * H:(h + 1) * H],
                            psum_y[g][:, h * H:(h + 1) * H])
                        yq[2 * g + h].dma_start(
                            y_d[g * P:(g + 1) * P,
                                w * W + h * H:w * W + (h + 1) * H],
                            y_sb[g][:, h * H:(h + 1) * H],
                        )
    nc.compile()
    return nc


_NC_CACHE = None
LAST_RESULTS = None  # BassKernelResults of the most recent kernel() call


def _get_nc():
    global _NC_CACHE
    if _NC_CACHE is None:
        _NC_CACHE = _build_nc()
    return _NC_CACHE


def kernel(x: np.ndarray, sos: np.ndarray) -> np.ndarray:
    x = np.asarray(x)
    orig_shape = x.shape
    orig_dtype = x.dtype
    THW, ZA, FTR, A4T = _build_matrices(np.asarray(sos, dtype=np.float64))

    xf = np.ascontiguousarray(x.reshape(NSIG, T), dtype=np.float32)
    ident = np.eye(P, dtype=np.float32)
    ctab = np.concatenate([THW, FTR], axis=1).astype(np.float16)
    ctab8 = np.concatenate(
        [ZA, A4T, np.zeros((NST, 2 * P), np.float32)], axis=1
    ).astype(np.float16)
    in_maps = [
        {"x": xf[c * SPC:(c + 1) * SPC], "ident": ident, "ctab": ctab,
         "ctab8": ctab8}
        for c in range(NCORES)
    ]
    nc = _get_nc()
    res = run_bass_kernel_spmd(nc, in_maps, core_ids=list(range(NCORES)))
    global LAST_RESULTS
    LAST_RESULTS = res
    y = np.concatenate([res.results[c]["y"] for c in range(NCORES)], axis=0)
    return y.reshape(orig_shape).astype(orig_dtype, copy=False)

